# revision 4
# baseline (speedup 1.0000x reference)
"""NodeAttention trn2 kernel, v4: natural-layout s_dep, no reshuffle.

Per tile [128 i, 128 j x 64 k] (f32, 4 MiB DMA):
  - DVE contiguous multiply by w_dep pattern -> bf16 sc (1 op/tile)
  - PE: one matmul per 8-j window (512 contiguous moving cols, bf16,
    identity stationary). The out AP [[8,8],[0,8],[1,8]] interleaves 8
    sub-accumulators per j at ADJACENT addresses (innermost stride 1 --
    required for correct PSUM multi-hit accumulation); each address
    accumulates its 8 k-contributions within the instruction.
  - per chunk: one strided DVE tensor_reduce folds the 8 interleaved
    sub-accumulators -> sd[i,j] in SBUF.
  - per i-block: e = (sd != 0) * exp(sd), 4 PE transposes of e (bf16),
    bf16 matmuls against [g*feat | g] give agg + softmax denominator.
"""
import sys

if "/opt/trn_rl_repo" not in sys.path:
    sys.path.insert(0, "/opt/trn_rl_repo")

import numpy as np
from contextlib import ExitStack

import concourse.bass as bass
from concourse import bacc
import concourse.mybir as mybir
import concourse.tile as tile
from concourse.bass_utils import run_bass_kernel_spmd

F32 = mybir.dt.float32
F32R = mybir.dt.float32r
BF16 = mybir.dt.bfloat16

N = 512
D = 768
DEP = 64
P = 128
NB = N // P
JT = 128            # j's per adj tile
NJC = N // JT
JW = 8              # j's per matmul window (8 x 64 = 512 moving cols)
NW = JT // JW       # 16 windows per tile
SUB = 8             # PSUM sub-planes (accumulate revisit spacing = SUB)

_CACHED = {}


def _build():
    nc = bacc.Bacc()
    adj = nc.dram_tensor("adj", [N, N * DEP], F32, kind="ExternalInput")
    feat = nc.dram_tensor("feat", [N, D], F32, kind="ExternalInput")
    aspf = nc.dram_tensor("aspf", [N], F32, kind="ExternalInput")
    ident = nc.dram_tensor("ident", [P, P], F32, kind="ExternalInput")
    wdep = nc.dram_tensor("wdep", [DEP], F32, kind="ExternalInput")
    wnbr = nc.dram_tensor("wnbr", [D], F32, kind="ExternalInput")
    out = nc.dram_tensor("out", [N, D], F32, kind="ExternalOutput")

    with ExitStack() as ctx:
        tc = ctx.enter_context(tile.TileContext(nc))
        const = ctx.enter_context(tc.tile_pool(name="const", bufs=1))
        rawp = ctx.enter_context(tc.tile_pool(name="rawp", bufs=3))
        scp = ctx.enter_context(tc.tile_pool(name="scp", bufs=2))
        work = ctx.enter_context(tc.tile_pool(name="work", bufs=2))
        opool = ctx.enter_context(tc.tile_pool(name="opool", bufs=1))
        sd_ps = ctx.enter_context(tc.tile_pool(name="sd_ps", bufs=2, space="PSUM"))
        tp_ps = ctx.enter_context(tc.tile_pool(name="tp_ps", bufs=1, space="PSUM"))
        agg_ps = ctx.enter_context(tc.tile_pool(name="agg_ps", bufs=1, space="PSUM"))

        # ---- prefetch first adj tiles so the stream queue leads ----
        raws_pre = []
        for jc in range(2):
            r0 = rawp.tile([P, JT * DEP], F32, tag="raw")
            nc.sync.dma_start(
                r0[:], adj[0:P, jc * JT * DEP:(jc + 1) * JT * DEP])
            raws_pre.append(r0)

        # ---- constants / small inputs ----
        w64 = const.tile([P, DEP], F32)
        wd_ap = wdep[:]
        nc.scalar.dma_start(
            w64[:],
            bass.AP(tensor=wd_ap.tensor, offset=wd_ap.offset,
                    ap=[[0, P]] + [list(d) for d in wd_ap.ap]),
        )
        wnbr_sb = const.tile([P, D], F32)
        wn_ap = wnbr[:]
        nc.scalar.dma_start(
            wnbr_sb[:],
            bass.AP(tensor=wn_ap.tensor, offset=wn_ap.offset,
                    ap=[[0, P]] + [list(d) for d in wn_ap.ap]),
        )
        identf = const.tile([P, P], F32)
        nc.scalar.dma_start(identf[:], ident[:, :])
        identb = const.tile([P, P], BF16)
        nc.vector.tensor_copy(identb[:], identf[:])

        # natural-layout repeated w: wrepn[p, j*DEP + k] = w_dep[k]
        wrepn = const.tile([P, JT * DEP], BF16)
        w64_ap = w64[:]
        wr_ap = wrepn[:]
        nc.vector.tensor_copy(
            bass.AP(tensor=wr_ap.tensor, offset=wr_ap.offset,
                    ap=[list(wr_ap.ap[0]), [DEP, JT], [1, DEP]]),
            bass.AP(tensor=w64_ap.tensor, offset=w64_ap.offset,
                    ap=[list(w64_ap.ap[0]), [0, JT], [1, DEP]]),
        )

        featp = []
        for b in range(NB):
            f = const.tile([P, D], F32, tag=f"featp{b}", name=f"featp{b}")
            nc.scalar.dma_start(f[:], feat[b * P:(b + 1) * P, :])
            featp.append(f)

        aspf_sb = const.tile([P, NB], F32)
        for b in range(NB):
            nc.scalar.dma_start(aspf_sb[:, b:b + 1], aspf[b * P:(b + 1) * P])

        snbr = const.tile([P, NB], F32)
        g = const.tile([P, NB], F32)
        featg = []
        for b in range(NB):
            fw = work.tile([P, D], F32, tag="fw")
            nc.vector.tensor_mul(fw[:], featp[b][:], wnbr_sb[:])
            nc.vector.tensor_reduce(
                snbr[:, b:b + 1], fw[:],
                axis=mybir.AxisListType.X, op=mybir.AluOpType.add,
            )
            nc.scalar.activation(
                g[:, b:b + 1], snbr[:, b:b + 1],
                mybir.ActivationFunctionType.Exp,
            )
            fg = const.tile([P, D + 1], BF16, tag=f"featg{b}", name=f"featg{b}")
            nc.scalar.mul(fg[:, 0:D], featp[b][:], g[:, b:b + 1])
            nc.scalar.copy(fg[:, D:D + 1], g[:, b:b + 1])
            featg.append(fg)

        # ---- main stream ----
        for ib in range(NB):
            sd_sb = work.tile([P, N], F32, tag="sd_sb")
            for jc in range(NJC):
                if ib == 0 and jc < len(raws_pre):
                    raw = raws_pre[jc]
                else:
                    raw = rawp.tile([P, JT * DEP], F32, tag="raw")
                    nc.sync.dma_start(
                        raw[:],
                        adj[ib * P:(ib + 1) * P,
                            jc * JT * DEP:(jc + 1) * JT * DEP],
                    )
                sc = scp.tile([P, JT * DEP], BF16, tag="sc")
                # ACT casts f32->bf16; DVE then multiplies in place at 2x
                nc.scalar.copy(sc[:], raw[:])
                nc.vector.tensor_mul(sc[:], sc[:], wrepn[:])

                # 8 sub-planes of [128, JT] for this chunk (2 PSUM banks)
                sd4 = sd_ps.tile([P, SUB * JT], F32, tag="sd4")
                sd4_ap = sd4[:]
                for w in range(NW):
                    rhs = bass.AP(
                        tensor=sc[:].tensor,
                        offset=sc[:].offset + w * JW * DEP,
                        ap=[list(sc[:].ap[0]),
                            [DEP, JW], [SUB, DEP // SUB], [1, SUB]],
                    )
                    out_ap = bass.AP(
                        tensor=sd4_ap.tensor,
                        offset=sd4_ap.offset + w * JW * SUB,
                        ap=[list(sd4_ap.ap[0]),
                            [SUB, JW], [0, DEP // SUB], [1, SUB]],
                    )
                    nc.tensor.matmul(
                        out_ap, identb[:], rhs, start=True, stop=True,
                    )
                # fold the interleaved sub-accumulators: one strided reduce
                sdj = sd_sb[:, jc * JT:(jc + 1) * JT]
                rv = bass.AP(tensor=sd4_ap.tensor, offset=sd4_ap.offset,
                             ap=[list(sd4_ap.ap[0]), [SUB, JT], [1, SUB]])
                nc.vector.tensor_reduce(
                    sdj.unsqueeze(2), rv,
                    axis=mybir.AxisListType.X, op=mybir.AluOpType.add,
                )

            # ---- per-i-block epilogue ----

            ex = work.tile([P, N], BF16, tag="ex")
            nc.scalar.activation(
                ex[:], sd_sb[:], mybir.ActivationFunctionType.Exp,
            )
            e2 = work.tile([P, N], BF16, tag="e2")
            nc.vector.scalar_tensor_tensor(
                e2[:], sd_sb[:], 0.0, ex[:],
                op0=mybir.AluOpType.not_equal, op1=mybir.AluOpType.mult,
            )
            tp = tp_ps.tile([P, N], BF16, tag="tp")
            for jb in range(NB):
                nc.tensor.transpose(
                    tp[:, jb * P:(jb + 1) * P],
                    e2[:, jb * P:(jb + 1) * P],
                    identb[:],
                )
            eT = work.tile([P, N], BF16, tag="eT")
            nc.scalar.copy(eT[:], tp[:])

            agg = agg_ps.tile([P, D + 1], F32, tag="agg")
            for jb in range(NB):
                for c0, c1 in ((0, 512), (512, D + 1)):
                    nc.tensor.matmul(
                        agg[:, c0:c1],
                        eT[:, jb * P:(jb + 1) * P],
                        featg[jb][:, c0:c1],
                        start=(jb == 0),
                        stop=(jb == NB - 1),
                    )

            den = opool.tile([P, 4], F32, tag="den")
            nc.vector.tensor_scalar(
                den[:, 0:1], agg[:, D:D + 1], 1e-30, None,
                op0=mybir.AluOpType.max,
            )
            nc.vector.tensor_scalar(
                den[:, 1:2], agg[:, D:D + 1], 0.0, None,
                op0=mybir.AluOpType.is_gt,
            )
            nc.vector.reciprocal(den[:, 2:3], den[:, 0:1])
            u = opool.tile([P, 3], F32, tag="u")
            nc.vector.tensor_mul(u[:, 0:1], den[:, 1:2], aspf_sb[:, ib:ib + 1])
            nc.vector.tensor_mul(u[:, 1:2], u[:, 0:1], den[:, 2:3])
            nc.vector.tensor_scalar(
                u[:, 2:3], u[:, 0:1], -1.0, 1.0,
                op0=mybir.AluOpType.mult, op1=mybir.AluOpType.add,
            )
            o1 = opool.tile([P, D], F32, tag="o1")
            nc.scalar.mul(o1[:], agg[:, 0:D], u[:, 1:2])
            o2 = opool.tile([P, D], F32, tag="o2")
            nc.scalar.mul(o2[:], featp[ib][:], u[:, 2:3])
            nc.vector.tensor_add(o1[:], o1[:], o2[:])
            nc.sync.dma_start(out[ib * P:(ib + 1) * P, :], o1[:])

    nc.finalize()
    return nc


def _get_nc():
    if "nc" not in _CACHED:
        _CACHED["nc"] = _build()
    return _CACHED["nc"]


def kernel(features, aspect_onehot, adj_matrix, w_att):
    features = np.ascontiguousarray(features, dtype=np.float32)
    adj_matrix = np.ascontiguousarray(adj_matrix, dtype=np.float32)
    w_att = np.asarray(w_att, dtype=np.float32)
    B = features.shape[0]

    ident = np.eye(P, dtype=np.float32)
    aspf = aspect_onehot.astype(np.float32)

    nc = _get_nc()
    in_maps = [
        {
            "adj": adj_matrix[b].reshape(N, N * DEP),
            "feat": features[b],
            "aspf": aspf[b],
            "ident": ident,
            "wdep": w_att[D:D + DEP].copy(),
            "wnbr": w_att[0:D].copy(),
        }
        for b in range(B)
    ]
    res = run_bass_kernel_spmd(nc, in_maps, list(range(B)))
    return np.stack([res.results[b]["out"] for b in range(B)], axis=0)
